# revision 22
# baseline (speedup 1.0000x reference)
"""BitLinear (ternary weight + int8 activation quant) Trainium2 kernel.

Math (matches the jax reference exactly up to fp32 rounding):
  w_scale = mean(|W|) + 1e-8                       (global scalar)
  w_q     = clip(round(W / w_scale), -1, 1)        (ternary)
  x_scale = clip(max|x| over features, 1e-8)       (per token)
  x_q     = clip(round(x * 127 / x_scale), -127, 127)
  y       = (x_q @ w_q.T) * (x_scale/127) * w_scale

Key facts used:
  * x_q in [-127,127] and w_q in {-1,0,1} are exactly representable in
    bf16; dot products accumulate integers < 2^24 so the fp32 PSUM
    accumulation is EXACT -> the big matmul runs at bf16 PE rate with
    integer-exact results.
  * round-to-nearest-even of |v| <= 2^22 is (v + 12582912.0) - 12582912.0
    in fp32 (one rounded add; done on the scalar engine as in*1+bias).
  * clip(round(q), -1, 1) == sign(round(q)) for integer round(q), so the
    whole weight ternarization is two scalar-engine activations.

Sharding: 8-way token parallel. Each core gets 1024 tokens and streams
the full weight (quantized on the fly). The |W|-mean partial sum is
computed from the FIRST weight block each core streams (the host hands
each core the 8 output-column blocks rotated so core c sees global
block c first) and all-reduced across cores (32B collective). The h=0
quarter of that first block stays resident in SBUF so quantization can
begin the moment the collective lands, with no re-read.

Pipeline-fill schedule (all on the sync-engine HWDGE FIFO, so program
order == issue order): [reduce block 16 chunks] [x tokens 0:511]
[block-0 h1..h3 re-read interleaved with x tokens 512:1023] [block 1]
[block 2] ... Output stores ride the gpsimd SWDGE ring so they never
head-of-line-block input loads.

The matmul emits y TRANSPOSED ([O, T] per core, lhsT = w_q); the host
gather transposes back and un-rotates the column blocks.
"""

import numpy as np

import concourse.bass as bass
import concourse.bass_isa as bass_isa
import concourse.mybir as mybir
import concourse.tile as tile
from concourse import bacc
from concourse import bass_utils

F32 = mybir.dt.float32
BF16 = mybir.dt.bfloat16
AX = mybir.AxisListType
OP = mybir.AluOpType
AF = mybir.ActivationFunctionType

MAGIC = 12582912.0  # 1.5 * 2^23: fp32 RNE-to-integer trick
QB = 127.0
EPS = 1e-8

N_CORES = 8
B_FULL, S_FULL, D_FULL, O_FULL = 4, 2048, 4096, 4096
T_FULL = B_FULL * S_FULL  # 8192 tokens


def _shapes(n_cores, T, D, O):
    P = 128
    PO = D // P
    TB = 128                      # x block: tokens per staged block
    n_tb = T // TB
    TH = min(512, T)              # matmul rhs free dim
    n_th = T // TH
    OB = min(256, O)              # w-quant block (out cols)
    n_ob = O // OB
    BPC = O // (n_cores * OB)     # blocks per core's 1/8 slice
    WB = min(8, PO)               # d-chunks per W dma
    G = PO // WB
    OW = min(128, OB)             # o-width per W dma
    H = OB // OW
    return dict(P=P, PO=PO, TB=TB, n_tb=n_tb, TH=TH, n_th=n_th, OB=OB,
                n_ob=n_ob, WB=WB, G=G, OW=OW, H=H, BPC=BPC)


def build_bitlinear(n_cores, T, D, O):
    S = _shapes(n_cores, T, D, O)
    P, PO, TB, n_tb = S["P"], S["PO"], S["TB"], S["n_tb"]
    TH, n_th, OB, n_ob = S["TH"], S["n_th"], S["OB"], S["n_ob"]
    WB, G, OW, H, BPC = S["WB"], S["G"], S["OW"], S["H"], S["BPC"]
    assert D % P == 0 and T % TB == 0 and O % OB == 0 and OW == P == TB
    n_wch = BPC * H               # h-groups in the phase-0 reduce slice

    nc = bacc.Bacc(
        "TRN2",
        target_bir_lowering=False,
        debug=False,
        enable_asserts=False,
        num_devices=n_cores,
    )
    # host-blocked layouts: every dma slice is one contiguous region
    xb = nc.dram_tensor("xb", [n_tb, P, PO, TB], F32, kind="ExternalInput").ap()
    wb = nc.dram_tensor(
        "wb", [n_ob, H, P, PO, OW], F32, kind="ExternalInput"
    ).ap()
    yT = nc.dram_tensor("y", [O, T], F32, kind="ExternalOutput").ap()

    with tile.TileContext(nc) as tc:
        with (
            tc.tile_pool(name="const", bufs=1) as cpool,
            tc.tile_pool(name="stage", bufs=3) as stage,
            tc.tile_pool(name="wres", bufs=1) as wresp,
            tc.tile_pool(name="wq", bufs=4) as wqp,
            tc.tile_pool(name="xq", bufs=1) as xqp,
            tc.tile_pool(name="acc", bufs=2) as accp,
            tc.tile_pool(name="outp", bufs=2) as outp,
            tc.tile_pool(name="pmm", bufs=4, space="PSUM") as pmm,
            tc.tile_pool(name="psm", bufs=1, space="PSUM") as psm,
            tc.tile_pool(name="dram", bufs=2, space="DRAM") as dram,
        ):
            # ---------------- constants / small scratch ----------------
            scratch = cpool.tile([P, 192], F32, name="scratch")
            nc.gpsimd.memset(scratch[:], 0.0)
            ones = scratch[:, 0:128]
            nc.gpsimd.memset(ones, 1.0)
            negm_bc = scratch[:, 133:134]
            nc.gpsimd.memset(negm_bc, -MAGIC)
            sums = scratch[:, 134 : 134 + n_wch]
            part128 = scratch[:, 128:129]
            zcol2 = scratch[:, 129:131]
            invsw_bc = scratch[:, 131:133]
            invs_bc = invsw_bc[:, 0:1]
            sw_bc = invsw_bc[:, 1:2]
            s_sb = scratch[0:1, 168:169]
            inv_sb = scratch[0:1, 169:170]
            sw_sb = scratch[0:1, 170:171]
            tot_sb = scratch[0:1, 172:180]   # [1,8] allreduce payload row
            part_sb = scratch[0:1, 180:188]  # [1,8] (col 0 = partial, rest 0)

            s_half = [
                cpool.tile([P, TH], F32, name=f"s_half{i}") for i in range(n_th)
            ]
            xq = xqp.tile([P, PO, T], BF16, name="xq")
            # resident h=0 quarter of the core's own (first) weight block
            wres = wresp.tile([P, PO, OW], F32, name="wres")

            # ------------- phase 0: w_scale partial + allreduce ---------
            # A dummy AllReduce with no data deps fires at t~0: it absorbs
            # the one-time collective bootstrap + entry-barrier skew
            # concurrently with the phase-0 DMAs, so the real AllReduce
            # below rides a warm channel.
            bb_d_in = dram.tile([1, 8], F32, name="bb_d_in")
            bb_d_out = dram.tile([1, 8], F32, name="bb_d_out")
            nc.gpsimd.collective_compute(
                "AllReduce",
                OP.add,
                replica_groups=[list(range(n_cores))],
                ins=[bb_d_in[:].opt()],
                outs=[bb_d_out[:].opt()],
            )
            # Stream the core's own 1/n_cores slice (local blocks
            # 0..BPC-1), one 2.1MB DMA per h-group (HWDGE triggers cost
            # ~0.6us each — keep them few). Block 0 h=0 lands in the
            # resident tile (reused for quantization later); the rest
            # pass through the shared staging pool and are re-read in
            # the main loop.
            for b in range(BPC):
                for h in range(H):
                    if b == 0 and h == 0:
                        st = wres[:]
                    else:
                        stt = stage.tile([P, PO, OW], F32, name="wst", tag="stage")
                        st = stt[:]
                    nc.sync.dma_start(st, wb[b, h])
                    nc.vector.tensor_reduce(
                        out=sums[:, b * H + h : b * H + h + 1],
                        in_=st,
                        axis=AX.XY,
                        op=OP.add,
                        apply_absolute_value=True,
                    )
            nc.vector.tensor_reduce(out=part128, in_=sums, axis=AX.X, op=OP.add)
            ps_tot = psm.tile([1, 1], F32, name="ps_tot", tag="psm1")
            nc.tensor.matmul(ps_tot[:], part128, ones[:, 0:1], start=True, stop=True)
            nc.vector.tensor_copy(out=part_sb[:, 0:1], in_=ps_tot[:])

            bb_in = dram.tile([1, 8], F32, name="bb_in")
            bb_out = dram.tile([1, 8], F32, name="bb_out")
            nc.sync.dma_start(bb_in[:], part_sb)
            nc.gpsimd.collective_compute(
                "AllReduce",
                OP.add,
                replica_groups=[list(range(n_cores))],
                ins=[bb_in[:].opt()],
                outs=[bb_out[:].opt()],
            )
            nc.sync.dma_start(tot_sb, bb_out[:])
            numel = float(n_cores * BPC * OB * D)
            nc.vector.tensor_scalar(
                s_sb, tot_sb[:, 0:1], 1.0 / numel, EPS, OP.mult, OP.add
            )
            nc.vector.reciprocal(inv_sb, s_sb)
            nc.vector.tensor_scalar(sw_sb, s_sb, 1.0 / QB, None, OP.mult)
            nc.vector.tensor_copy(out=zcol2[0:1, 0:1], in_=inv_sb)
            nc.vector.tensor_copy(out=zcol2[0:1, 1:2], in_=sw_sb)
            ps_b = psm.tile([P, 2], F32, name="ps_b", tag="psm2")
            nc.tensor.matmul(ps_b[:], ones, zcol2, start=True, stop=True)
            nc.vector.tensor_copy(out=invsw_bc, in_=ps_b[:])

            # ---------------- x pass: absmax + quantize (single read) ----
            def x_block(tb):
                t0 = tb * TB
                th_i = t0 // TH
                s_blk = s_half[th_i][:, t0 - th_i * TH : t0 - th_i * TH + TB]
                st = stage.tile([P, PO, TB], F32, name="xst", tag="stage")
                nc.sync.dma_start(st[:], xb[tb])
                absm = accp.tile([P, TB], F32, name="absm", tag="absm")
                # absmax over the PO dim (strided-inner view)
                nc.vector.tensor_reduce(
                    out=absm[:],
                    in_=st.rearrange("p a b -> p b a"),
                    axis=AX.X,
                    op=OP.max,
                    apply_absolute_value=True,
                )
                # absmax over partitions -> every partition holds s_token
                nc.gpsimd.partition_all_reduce(
                    s_blk, absm[:], channels=P,
                    reduce_op=bass_isa.ReduceOp.absmax,
                )
                # r = 127/s
                r_blk = accp.tile([P, TB], F32, name="r_blk", tag="rblk")
                nc.vector.reciprocal(r_blk[:], s_blk)
                nc.vector.tensor_scalar(r_blk[:], r_blk[:], QB, None, OP.mult)
                # x * r  (DVE, in place, fp32)
                nc.vector.tensor_tensor(
                    st[:],
                    st[:],
                    r_blk[:, None, :].to_broadcast((P, PO, TB)),
                    OP.mult,
                )
                # round via (+M, -M) fused DVE op -> bf16
                nc.vector.tensor_scalar(
                    xq[:, :, t0 : t0 + TB], st[:],
                    MAGIC, MAGIC, OP.add, OP.subtract,
                )

            # fold w_scale/127 into the per-token scales so the psum
            # evacuation is a single tensor_tensor
            def fold_half(th_i):
                nc.vector.tensor_scalar(
                    s_half[th_i][:], s_half[th_i][:], sw_bc, None, OP.mult
                )

            # first token half: needed by the first matmuls
            for tb in range(n_tb // 2):
                x_block(tb)
            fold_half(0)

            # ---------------- main: quantize W + matmul ----------------
            def quant_chunk(src, wq_t, h):
                # q + MAGIC (the add rounds q to integer k via RNE)
                nc.scalar.activation(src, src, AF.Copy, bias=MAGIC, scale=invs_bc)
                # wq = sign(k) = clip(round(q), -1, 1) -> bf16
                nc.scalar.activation(
                    wq_t[:, :, h * OW : (h + 1) * OW],
                    src,
                    AF.Sign,
                    bias=negm_bc,
                    scale=1.0,
                )

            def mm_group(wq_t, ob_i, oc, th):
                ps = pmm.tile([P, TH], F32, name="ps", tag="ps")
                for po in range(PO):
                    nc.tensor.matmul(
                        ps[:],
                        wq_t[:, po, oc * P : (oc + 1) * P],
                        xq[:, po, th * TH : (th + 1) * TH],
                        start=(po == 0),
                        stop=(po == PO - 1),
                    )
                osb = outp.tile([P, TH], F32, name="osb")
                orow = ob_i * OB + oc * P
                # y = psum * (s_token * s_w/127)   (sw pre-folded)
                nc.vector.tensor_tensor(osb[:], ps[:], s_half[th][:], OP.mult)
                # store on the gpsimd SWDGE ring: never blocks input loads
                nc.gpsimd.dma_start(
                    yT[orow : orow + P, th * TH : (th + 1) * TH], osb[:]
                )

            wq_tiles = {}

            def quant_block(ob_i, h_list):
                if ob_i not in wq_tiles:
                    wq_tiles[ob_i] = wqp.tile([P, PO, OB], BF16, name="wq", tag="wq")
                wq_t = wq_tiles[ob_i]
                for h in h_list:
                    if ob_i == 0 and h == 0:
                        quant_chunk(wres[:], wq_t, h)
                    else:
                        stt = stage.tile([P, PO, OW], F32, name="wst", tag="stage")
                        nc.sync.dma_start(stt[:], wb[ob_i, h])
                        quant_chunk(stt[:], wq_t, h)
                return wq_t

            # block 0: h0 quantizes from the resident tile (no DMA); the
            # re-reads of the rest of the phase-0 slice interleave with
            # the second x half so the first matmuls and the th=1
            # operands arrive together.
            wq0 = quant_block(0, [0, 1])
            x_block(n_tb // 2)
            x_block(n_tb // 2 + 1)
            wq1 = quant_block(1, list(range(H)))
            x_block(n_tb // 2 + 2)
            x_block(n_tb // 2 + 3)
            fold_half(1)

            for oc in range(OB // P):
                mm_group(wq0, 0, oc, 0)
            for oc in range(OB // P):
                mm_group(wq0, 0, oc, 1)

            pending = None
            for ob_i in range(1, n_ob):
                wq_t = quant_block(ob_i, list(range(H)))
                for oc in range(OB // P):
                    mm_group(wq_t, ob_i, oc, 0)
                if pending is not None:
                    pwq, pob = pending
                    for th in range(1, n_th):
                        for oc in range(OB // P):
                            mm_group(pwq, pob, oc, th)
                pending = (wq_t, ob_i)
            pwq, pob = pending
            for th in range(1, n_th):
                for oc in range(OB // P):
                    mm_group(pwq, pob, oc, th)

    nc.compile()
    return nc


_NC_CACHE = {}


def _get_nc(n_cores, T, D, O):
    key = (n_cores, T, D, O)
    if key not in _NC_CACHE:
        _NC_CACHE[key] = build_bitlinear(n_cores, T, D, O)
    return _NC_CACHE[key]


def make_in_maps(x, weight, n_cores):
    """Host-side sharding + blocking (layout only, no math)."""
    T_total = int(np.prod(x.shape[:-1]))
    D = x.shape[-1]
    O = weight.shape[0]
    Tc = T_total // n_cores
    S = _shapes(n_cores, Tc, D, O)
    P, PO, TB, n_tb = S["P"], S["PO"], S["TB"], S["n_tb"]
    OB, n_ob, WB, G, OW, H = S["OB"], S["n_ob"], S["WB"], S["G"], S["OW"], S["H"]

    x2d = x.reshape(T_total, D)
    # wb[ob, h, pi, j, o] = W[ob*OB + h*OW + o, j*P + pi]
    wT = weight.reshape(n_ob, H, OW, PO, P)  # [ob, h, o, j, pi]
    wb = np.ascontiguousarray(wT.transpose(0, 1, 4, 3, 2))
    in_maps = []
    for c in range(n_cores):
        xc = x2d[c * Tc : (c + 1) * Tc]  # [Tc, D]
        # xb[tb, pi, po, t] = xc[tb*TB + t, po*P + pi]
        xblk = np.ascontiguousarray(
            xc.reshape(n_tb, TB, PO, P).transpose(0, 3, 2, 1)
        )
        # rotate the column blocks so core c streams its own 1/8 first
        BPC = S["BPC"]
        rot = [(BPC * c + i) % n_ob for i in range(n_ob)]
        wbc = np.ascontiguousarray(wb[rot])
        in_maps.append({"xb": xblk, "wb": wbc})
    return in_maps


def run_on_hw(x, weight, n_cores=N_CORES, trace=False, **kw):
    T_total = int(np.prod(x.shape[:-1]))
    D = x.shape[-1]
    O = weight.shape[0]
    Tc = T_total // n_cores
    S = _shapes(n_cores, Tc, D, O)
    OB, n_ob, BPC = S["OB"], S["n_ob"], S["BPC"]
    nc = _get_nc(n_cores, Tc, D, O)
    in_maps = make_in_maps(x, weight, n_cores)
    res = bass_utils.run_bass_kernel_spmd(
        nc, in_maps, core_ids=list(range(n_cores)), trace=trace, **kw
    )
    parts = []
    for c in range(n_cores):
        yc = res.results[c]["y"]  # [O, Tc], rows in rotated block order
        un = np.empty_like(yc)
        for i in range(n_ob):
            gi = (BPC * c + i) % n_ob
            un[gi * OB : (gi + 1) * OB] = yc[i * OB : (i + 1) * OB]
        parts.append(un.T)
    y = np.ascontiguousarray(np.concatenate(parts, axis=0)).reshape(
        *x.shape[:-1], O
    )
    return y.astype(np.float32, copy=False), res


def kernel(x, weight):
    y, _ = run_on_hw(
        np.asarray(x, dtype=np.float32), np.asarray(weight, dtype=np.float32)
    )
    return y


# revision 25
# speedup vs baseline: 1.0157x; 1.0157x over previous
"""BitLinear (ternary weight + int8 activation quant) Trainium2 kernel.

Math (matches the jax reference exactly up to fp32 rounding):
  w_scale = mean(|W|) + 1e-8                       (global scalar)
  w_q     = clip(round(W / w_scale), -1, 1)        (ternary)
  x_scale = clip(max|x| over features, 1e-8)       (per token)
  x_q     = clip(round(x * 127 / x_scale), -127, 127)
  y       = (x_q @ w_q.T) * (x_scale/127) * w_scale

Key facts used:
  * x_q in [-127,127] and w_q in {-1,0,1} are exactly representable in
    bf16; dot products accumulate integers < 2^24 so the fp32 PSUM
    accumulation is EXACT -> the big matmul runs at bf16 PE rate with
    integer-exact results.
  * round-to-nearest-even of |v| <= 2^22 is (v + 12582912.0) - 12582912.0
    in fp32 (one rounded add; done on the scalar engine as in*1+bias).
  * clip(round(q), -1, 1) == sign(round(q)) for integer round(q), so the
    whole weight ternarization is two scalar-engine activations.

Sharding: 8-way token parallel. Each core gets 1024 tokens and streams
the full weight (quantized on the fly). The |W|-mean partial sum is
computed from the FIRST weight block each core streams (the host hands
each core the 8 output-column blocks rotated so core c sees global
block c first) and all-reduced across cores (32B collective). The h=0
quarter of that first block stays resident in SBUF so quantization can
begin the moment the collective lands, with no re-read.

Pipeline-fill schedule (all on the sync-engine HWDGE FIFO, so program
order == issue order): [reduce block 16 chunks] [x tokens 0:511]
[block-0 h1..h3 re-read interleaved with x tokens 512:1023] [block 1]
[block 2] ... Output stores ride the gpsimd SWDGE ring so they never
head-of-line-block input loads.

The matmul emits y TRANSPOSED ([O, T] per core, lhsT = w_q); the host
gather transposes back and un-rotates the column blocks.
"""

import numpy as np

import concourse.bass as bass
import concourse.bass_isa as bass_isa
import concourse.mybir as mybir
import concourse.tile as tile
from concourse import bacc
from concourse import bass_utils

F32 = mybir.dt.float32
BF16 = mybir.dt.bfloat16
AX = mybir.AxisListType
OP = mybir.AluOpType
AF = mybir.ActivationFunctionType

MAGIC = 12582912.0  # 1.5 * 2^23: fp32 RNE-to-integer trick
QB = 127.0
EPS = 1e-8

N_CORES = 8
B_FULL, S_FULL, D_FULL, O_FULL = 4, 2048, 4096, 4096
T_FULL = B_FULL * S_FULL  # 8192 tokens


def _shapes(n_cores, T, D, O):
    P = 128
    PO = D // P
    TB = 128                      # x block: tokens per staged block
    n_tb = T // TB
    TH = min(512, T)              # matmul rhs free dim
    n_th = T // TH
    OB = min(256, O)              # w-quant block (out cols)
    n_ob = O // OB
    BPC = O // (n_cores * OB)     # blocks per core's 1/8 slice
    WB = min(8, PO)               # d-chunks per W dma
    G = PO // WB
    OW = min(128, OB)             # o-width per W dma
    H = OB // OW
    return dict(P=P, PO=PO, TB=TB, n_tb=n_tb, TH=TH, n_th=n_th, OB=OB,
                n_ob=n_ob, WB=WB, G=G, OW=OW, H=H, BPC=BPC)


def build_bitlinear(n_cores, T, D, O):
    S = _shapes(n_cores, T, D, O)
    P, PO, TB, n_tb = S["P"], S["PO"], S["TB"], S["n_tb"]
    TH, n_th, OB, n_ob = S["TH"], S["n_th"], S["OB"], S["n_ob"]
    WB, G, OW, H, BPC = S["WB"], S["G"], S["OW"], S["H"], S["BPC"]
    assert D % P == 0 and T % TB == 0 and O % OB == 0 and OW == P == TB
    n_wch = BPC * H               # h-groups in the phase-0 reduce slice

    nc = bacc.Bacc(
        "TRN2",
        target_bir_lowering=False,
        debug=False,
        enable_asserts=False,
        num_devices=n_cores,
    )
    # host-blocked layouts: every dma slice is one contiguous region
    xb = nc.dram_tensor("xb", [n_tb, P, PO, TB], F32, kind="ExternalInput").ap()
    wb = nc.dram_tensor(
        "wb", [n_ob, H, P, PO, OW], F32, kind="ExternalInput"
    ).ap()
    yT = nc.dram_tensor("y", [O, T], F32, kind="ExternalOutput").ap()

    with tile.TileContext(nc) as tc:
        with (
            tc.tile_pool(name="const", bufs=1) as cpool,
            tc.tile_pool(name="stage", bufs=3) as stage,
            tc.tile_pool(name="wres", bufs=1) as wresp,
            tc.tile_pool(name="wq", bufs=4) as wqp,
            tc.tile_pool(name="xq", bufs=1) as xqp,
            tc.tile_pool(name="acc", bufs=2) as accp,
            tc.tile_pool(name="outp", bufs=2) as outp,
            tc.tile_pool(name="pmm", bufs=4, space="PSUM") as pmm,
            tc.tile_pool(name="psm", bufs=1, space="PSUM") as psm,
            tc.tile_pool(name="dram", bufs=2, space="DRAM") as dram,
        ):
            # ---------------- constants / small scratch ----------------
            scratch = cpool.tile([P, 192], F32, name="scratch")
            nc.gpsimd.memset(scratch[:], 0.0)
            ones = scratch[:, 0:128]
            nc.gpsimd.memset(ones, 1.0)
            negm_bc = scratch[:, 133:134]
            nc.gpsimd.memset(negm_bc, -MAGIC)
            sums = scratch[:, 134 : 134 + n_wch]
            part128 = scratch[:, 128:129]
            zcol2 = scratch[:, 129:131]
            invsw_bc = scratch[:, 131:133]
            invs_bc = invsw_bc[:, 0:1]
            sw_bc = invsw_bc[:, 1:2]
            s_sb = scratch[0:1, 168:169]
            inv_sb = scratch[0:1, 169:170]
            sw_sb = scratch[0:1, 170:171]
            tot_sb = scratch[0:1, 172:180]   # [1,8] allreduce payload row
            part_sb = scratch[0:1, 180:188]  # [1,8] (col 0 = partial, rest 0)

            s_half = [
                cpool.tile([P, TH], F32, name=f"s_half{i}") for i in range(n_th)
            ]
            xq = xqp.tile([P, PO, T], BF16, name="xq")
            # resident h=0 quarter of the core's own (first) weight block
            wres = wresp.tile([P, PO, OW], F32, name="wres")

            # ------------- phase 0: w_scale partial + allreduce ---------
            # Stream the core's own 1/n_cores slice (local blocks
            # 0..BPC-1), one 2.1MB DMA per h-group (HWDGE triggers cost
            # ~0.6us each — keep them few). Block 0 h=0 lands in the
            # resident tile (reused for quantization later); the rest
            # pass through the shared staging pool and are re-read in
            # the main loop.
            for b in range(BPC):
                for h in range(H):
                    if b == 0 and h == 0:
                        st = wres[:]
                    else:
                        stt = stage.tile([P, PO, OW], F32, name="wst", tag="stage")
                        st = stt[:]
                    nc.sync.dma_start(st, wb[b, h])
                    nc.vector.tensor_reduce(
                        out=sums[:, b * H + h : b * H + h + 1],
                        in_=st,
                        axis=AX.XY,
                        op=OP.add,
                        apply_absolute_value=True,
                    )
            nc.vector.tensor_reduce(out=part128, in_=sums, axis=AX.X, op=OP.add)
            ps_tot = psm.tile([1, 1], F32, name="ps_tot", tag="psm1")
            nc.tensor.matmul(ps_tot[:], part128, ones[:, 0:1], start=True, stop=True)
            nc.vector.tensor_copy(out=part_sb[:, 0:1], in_=ps_tot[:])

            bb_in = dram.tile([1, 8], F32, name="bb_in")
            bb_out = dram.tile([1, 8], F32, name="bb_out")
            nc.sync.dma_start(bb_in[:], part_sb)
            nc.gpsimd.collective_compute(
                "AllReduce",
                OP.add,
                replica_groups=[list(range(n_cores))],
                ins=[bb_in[:].opt()],
                outs=[bb_out[:].opt()],
            )
            nc.sync.dma_start(tot_sb, bb_out[:])
            numel = float(n_cores * BPC * OB * D)
            nc.vector.tensor_scalar(
                s_sb, tot_sb[:, 0:1], 1.0 / numel, EPS, OP.mult, OP.add
            )
            nc.vector.reciprocal(inv_sb, s_sb)
            nc.vector.tensor_scalar(sw_sb, s_sb, 1.0 / QB, None, OP.mult)
            nc.vector.tensor_copy(out=zcol2[0:1, 0:1], in_=inv_sb)
            nc.vector.tensor_copy(out=zcol2[0:1, 1:2], in_=sw_sb)
            ps_b = psm.tile([P, 2], F32, name="ps_b", tag="psm2")
            nc.tensor.matmul(ps_b[:], ones, zcol2, start=True, stop=True)
            nc.vector.tensor_copy(out=invsw_bc, in_=ps_b[:])

            # ---------------- x pass: absmax + quantize (single read) ----
            def x_block(tb):
                t0 = tb * TB
                th_i = t0 // TH
                s_blk = s_half[th_i][:, t0 - th_i * TH : t0 - th_i * TH + TB]
                st = stage.tile([P, PO, TB], F32, name="xst", tag="stage")
                nc.sync.dma_start(st[:], xb[tb])
                absm = accp.tile([P, TB], F32, name="absm", tag="absm")
                # absmax over the PO dim (strided-inner view)
                nc.vector.tensor_reduce(
                    out=absm[:],
                    in_=st.rearrange("p a b -> p b a"),
                    axis=AX.X,
                    op=OP.max,
                    apply_absolute_value=True,
                )
                # absmax over partitions -> every partition holds s_token
                nc.gpsimd.partition_all_reduce(
                    s_blk, absm[:], channels=P,
                    reduce_op=bass_isa.ReduceOp.absmax,
                )
                # r = 127/s
                r_blk = accp.tile([P, TB], F32, name="r_blk", tag="rblk")
                nc.vector.reciprocal(r_blk[:], s_blk)
                nc.vector.tensor_scalar(r_blk[:], r_blk[:], QB, None, OP.mult)
                # x * r  (DVE, in place, fp32)
                nc.vector.tensor_tensor(
                    st[:],
                    st[:],
                    r_blk[:, None, :].to_broadcast((P, PO, TB)),
                    OP.mult,
                )
                # round via (+M, -M) fused DVE op -> bf16
                nc.vector.tensor_scalar(
                    xq[:, :, t0 : t0 + TB], st[:],
                    MAGIC, MAGIC, OP.add, OP.subtract,
                )

            # fold w_scale/127 into the per-token scales so the psum
            # evacuation is a single tensor_tensor. Runs on gpsimd (idle
            # engine) and is emitted AFTER all x blocks: a DVE placement
            # mid-x-loop would stall the DVE queue on the collective.
            def fold_half(th_i):
                nc.gpsimd.tensor_scalar(
                    s_half[th_i][:], s_half[th_i][:], sw_bc, None, OP.mult
                )

            # first token half: needed by the first matmuls
            for tb in range(n_tb // 2):
                x_block(tb)

            # ---------------- main: quantize W + matmul ----------------
            def quant_chunk(src, wq_t, h):
                # q + MAGIC (the add rounds q to integer k via RNE)
                nc.scalar.activation(src, src, AF.Copy, bias=MAGIC, scale=invs_bc)
                # wq = sign(k) = clip(round(q), -1, 1) -> bf16
                nc.scalar.activation(
                    wq_t[:, :, h * OW : (h + 1) * OW],
                    src,
                    AF.Sign,
                    bias=negm_bc,
                    scale=1.0,
                )

            def mm_group(wq_t, ob_i, oc, th):
                ps = pmm.tile([P, TH], F32, name="ps", tag="ps")
                for po in range(PO):
                    nc.tensor.matmul(
                        ps[:],
                        wq_t[:, po, oc * P : (oc + 1) * P],
                        xq[:, po, th * TH : (th + 1) * TH],
                        start=(po == 0),
                        stop=(po == PO - 1),
                    )
                osb = outp.tile([P, TH], F32, name="osb")
                orow = ob_i * OB + oc * P
                # y = psum * (s_token * s_w/127)   (sw pre-folded)
                nc.vector.tensor_tensor(osb[:], ps[:], s_half[th][:], OP.mult)
                # store on the gpsimd SWDGE ring: never blocks input loads
                nc.gpsimd.dma_start(
                    yT[orow : orow + P, th * TH : (th + 1) * TH], osb[:]
                )

            wq_tiles = {}

            def quant_block(ob_i, h_list):
                if ob_i not in wq_tiles:
                    wq_tiles[ob_i] = wqp.tile([P, PO, OB], BF16, name="wq", tag="wq")
                wq_t = wq_tiles[ob_i]
                for h in h_list:
                    if ob_i == 0 and h == 0:
                        quant_chunk(wres[:], wq_t, h)
                    else:
                        stt = stage.tile([P, PO, OW], F32, name="wst", tag="stage")
                        nc.sync.dma_start(stt[:], wb[ob_i, h])
                        quant_chunk(stt[:], wq_t, h)
                return wq_t

            # block 0: h0 quantizes from the resident tile (no DMA); the
            # re-reads of the rest of the phase-0 slice interleave with
            # the second x half so the first matmuls and the th=1
            # operands arrive together.
            wq0 = quant_block(0, [0, 1])
            x_block(n_tb // 2)
            x_block(n_tb // 2 + 1)
            wq1 = quant_block(1, list(range(H)))
            x_block(n_tb // 2 + 2)
            x_block(n_tb // 2 + 3)
            fold_half(0)
            fold_half(1)

            for oc in range(OB // P):
                mm_group(wq0, 0, oc, 0)
            for oc in range(OB // P):
                mm_group(wq0, 0, oc, 1)

            pending = None
            for ob_i in range(1, n_ob):
                wq_t = (
                    wq1 if ob_i == 1 else quant_block(ob_i, list(range(H)))
                )
                for oc in range(OB // P):
                    mm_group(wq_t, ob_i, oc, 0)
                if pending is not None:
                    pwq, pob = pending
                    for th in range(1, n_th):
                        for oc in range(OB // P):
                            mm_group(pwq, pob, oc, th)
                pending = (wq_t, ob_i)
            pwq, pob = pending
            for th in range(1, n_th):
                for oc in range(OB // P):
                    mm_group(pwq, pob, oc, th)

    nc.compile()
    return nc


_NC_CACHE = {}


def _get_nc(n_cores, T, D, O):
    key = (n_cores, T, D, O)
    if key not in _NC_CACHE:
        _NC_CACHE[key] = build_bitlinear(n_cores, T, D, O)
    return _NC_CACHE[key]


def make_in_maps(x, weight, n_cores):
    """Host-side sharding + blocking (layout only, no math)."""
    T_total = int(np.prod(x.shape[:-1]))
    D = x.shape[-1]
    O = weight.shape[0]
    Tc = T_total // n_cores
    S = _shapes(n_cores, Tc, D, O)
    P, PO, TB, n_tb = S["P"], S["PO"], S["TB"], S["n_tb"]
    OB, n_ob, WB, G, OW, H = S["OB"], S["n_ob"], S["WB"], S["G"], S["OW"], S["H"]

    x2d = x.reshape(T_total, D)
    # wb[ob, h, pi, j, o] = W[ob*OB + h*OW + o, j*P + pi]
    wT = weight.reshape(n_ob, H, OW, PO, P)  # [ob, h, o, j, pi]
    wb = np.ascontiguousarray(wT.transpose(0, 1, 4, 3, 2))
    in_maps = []
    for c in range(n_cores):
        xc = x2d[c * Tc : (c + 1) * Tc]  # [Tc, D]
        # xb[tb, pi, po, t] = xc[tb*TB + t, po*P + pi]
        xblk = np.ascontiguousarray(
            xc.reshape(n_tb, TB, PO, P).transpose(0, 3, 2, 1)
        )
        # rotate the column blocks so core c streams its own 1/8 first
        BPC = S["BPC"]
        rot = [(BPC * c + i) % n_ob for i in range(n_ob)]
        wbc = np.ascontiguousarray(wb[rot])
        in_maps.append({"xb": xblk, "wb": wbc})
    return in_maps


def run_on_hw(x, weight, n_cores=N_CORES, trace=False, **kw):
    T_total = int(np.prod(x.shape[:-1]))
    D = x.shape[-1]
    O = weight.shape[0]
    Tc = T_total // n_cores
    S = _shapes(n_cores, Tc, D, O)
    OB, n_ob, BPC = S["OB"], S["n_ob"], S["BPC"]
    nc = _get_nc(n_cores, Tc, D, O)
    in_maps = make_in_maps(x, weight, n_cores)
    res = bass_utils.run_bass_kernel_spmd(
        nc, in_maps, core_ids=list(range(n_cores)), trace=trace, **kw
    )
    parts = []
    for c in range(n_cores):
        yc = res.results[c]["y"]  # [O, Tc], rows in rotated block order
        un = np.empty_like(yc)
        for i in range(n_ob):
            gi = (BPC * c + i) % n_ob
            un[gi * OB : (gi + 1) * OB] = yc[i * OB : (i + 1) * OB]
        parts.append(un.T)
    y = np.ascontiguousarray(np.concatenate(parts, axis=0)).reshape(
        *x.shape[:-1], O
    )
    return y.astype(np.float32, copy=False), res


def kernel(x, weight):
    y, _ = run_on_hw(
        np.asarray(x, dtype=np.float32), np.asarray(weight, dtype=np.float32)
    )
    return y
